# revision 1
# baseline (speedup 1.0000x reference)
"""Causal attention block (LN -> QKV -> causal MHA -> out-proj) on 8 trn2
NeuronCores via Bass/Tile.

Sharding: core c handles batch b=c//2 and head-group g=c%2 (8 of 16 heads).
Data parallel over batch, tensor parallel over heads; the out-proj partial
sums (2 per batch) are reduced on the host during the gather, so the device
program needs no collectives and is pure SPMD.

Per-core layout choices:
  - x arrives host-transposed (d-major) so the QKV contraction has d on
    partitions with no on-device transpose.
  - LayerNorm is folded: gamma into W (host), beta and the mean correction
    enter each QKV accumulation as a K=2 seed matmul (rank-2 term), and the
    rstd scaling is a single elementwise multiply with a PE-broadcast row.
  - Scores are computed transposed (S^T = K^T.T @ Q^T per 128-ktok chunk)
    with 2 heads row-packed on the PE (K=64 each); exp runs on ACT straight
    out of PSUM with the 1/sqrt(dh) scale folded into the activation; the
    causal mask is a 0/1 multiply applied only to the 4 diagonal chunks per
    query tile (strictly-upper chunks are never computed).
  - The softmax denominator is obtained for free as a 65th ones-column of V
    in the P@V matmul; O^T is normalized with a reciprocal broadcast and fed
    (via a small DRAM scratch) as the stationary operand of the out-proj.
  - All matmuls use float32r (full-rate fp32 on trn2 for moving dim >= 256).
"""

import numpy as np

import concourse.bass as bass
import concourse.mybir as mybir
import concourse.tile as tile_mod

# ----------------------------------------------------------------------------
# Workaround for this walrus build rejecting instructions that carry more than
# MAX_WAITS semaphore waits ("Too many sync wait commands" in CoreV3GenImpl
# setupSyncWait — hit on Drain and Matmult/S3_LW encodings). Split excess
# waits onto single-wait NOP carrier instructions emitted just before the
# original instruction on the same engine: program order on the sequencer
# makes this semantically identical (waits are AND conditions).
# ----------------------------------------------------------------------------
_MAX_WAITS = 1
_orig_add_instruction = tile_mod.TileContext._add_instruction
_carrier_id = [0]


def _split_waits_add_instruction(self, inst):
    si = inst.sync_info
    if (
        si is not None
        and si.on_wait
        and len(si.on_wait) > _MAX_WAITS
        and inst.engine != mybir.EngineType.Unassigned
    ):
        waits = list(si.on_wait)
        keep = waits[-_MAX_WAITS:]
        for w in waits[:-_MAX_WAITS]:
            _carrier_id[0] += 1
            nop = mybir.InstNoOp(name=f"I-waitc-{_carrier_id[0]}")
            nop.engine = inst.engine
            nop.sync_info = mybir.SyncInfo(on_wait=[w], on_update=[])
            _orig_add_instruction(self, nop)
        inst.sync_info = mybir.SyncInfo(
            on_wait=keep,
            on_update=list(si.on_update) if si.on_update else [],
        )
    _orig_add_instruction(self, inst)


tile_mod.TileContext._add_instruction = _split_waits_add_instruction

from concourse.vector_clock import ScopedClock


def _patched_drain_and_barrier(self, tick_clock, wait_clock):
    # Same wait-splitting for the TileContext exit drain, which is emitted
    # after lowering (outside _add_instruction).
    nc = self.nc
    carrier = nc.sync.nop(nofuse=True)
    wait_clock.add_sem_waits(carrier.ins, ScopedClock({None: tick_clock.global_clock}))
    si = carrier.ins.sync_info
    waits = list(si.on_wait) if si is not None and si.on_wait else []
    if len(waits) > _MAX_WAITS:
        carrier.ins.sync_info = mybir.SyncInfo(
            on_wait=waits[:_MAX_WAITS],
            on_update=list(si.on_update) if si.on_update else [],
        )
        rest = waits[_MAX_WAITS:]
        while rest:
            extra = nc.sync.nop(nofuse=True)
            extra.ins.sync_info = mybir.SyncInfo(
                on_wait=rest[:_MAX_WAITS], on_update=[])
            rest = rest[_MAX_WAITS:]

    nc.sync.drain()
    nc.all_engine_barrier()
    assert self.sems is not None
    popped = nc._tile_sem_poison_stack.pop()
    assert popped is self._sem_poison
    nc.clear_and_free_semaphores(list(self.sems.allocated().values()))
    nc.all_engine_barrier()


tile_mod.TileContext._drain_and_barrier = _patched_drain_and_barrier

# ----------------------------------------------------------------------------

F32 = mybir.dt.float32
F32R = mybir.dt.float32r
BF16 = mybir.dt.bfloat16
ALU = mybir.AluOpType
ACT_EXP = mybir.ActivationFunctionType.Exp
ACT_SQRT = mybir.ActivationFunctionType.Sqrt
U32 = mybir.dt.uint32
ONE_BITS = int(np.float32(1.0).view(np.uint32))

B = 4
TOK = 2048
DIM = 1024
HEADS = 16
DH = 64
HEADS_PC = 8          # heads per core
INNER_PC = HEADS_PC * DH  # 512
NPAIRS = HEADS_PC // 2
QT = 512              # query tile (matmul moving dim)
KC = 128              # key-token chunk (contraction tile)
EPS = 1e-5
SCALE = DH ** -0.5


def _r(ap):
    """View an fp32 AP as float32r for full-rate PE matmuls."""
    return ap.bitcast(F32R)


def _bcast(ap, parts):
    """Partition-broadcast AP (stride-0 leading dim) for DMA reads of DRAM."""
    return bass.AP(tensor=ap.tensor, offset=ap.offset, ap=[[0, parts]] + list(ap.ap))


def build_program(tok=TOK):
    ntt = tok // KC      # 128-token tiles
    nqt = tok // QT      # query tiles
    nkd = DIM // 128     # d-contraction chunks

    nc = bass.Bass()
    xT = nc.declare_dram_parameter("xT", [DIM, tok], F32R, isOutput=False)
    w = nc.declare_dram_parameter("w", [DIM, 3 * INNER_PC], F32R, isOutput=False)
    seed = nc.declare_dram_parameter("seed", [2, 3 * INNER_PC], F32R, isOutput=False)
    wo = nc.declare_dram_parameter("wo", [INNER_PC, DIM], F32R, isOutput=False)
    masks = nc.declare_dram_parameter("masks", [4, KC, QT], BF16, isOutput=False)
    out = nc.declare_dram_parameter("out", [tok, DIM], F32, isOutput=True)
    oTs = nc.dram_tensor("oT_scratch", [NPAIRS, 128, tok], F32R)

    with tile_mod.TileContext(nc) as tc, nc.allow_low_precision(
            "f32r-tagged operand tiles; all matmul accumulation stays fp32 PSUM"):
        with (
            tc.tile_pool(name="const", bufs=1) as const,
            tc.tile_pool(name="vpool", bufs=ntt) as vpool,
            tc.tile_pool(name="qkT", bufs=2) as qkp,
        ):
            # ---------------- constants ----------------
            ones_row = const.tile([1, 128], F32R, tag="ones_row")
            nc.vector.memset(ones_row.bitcast(U32), ONE_BITS)
            ones_col = const.tile([128, 1], F32R, tag="ones_col")
            nc.vector.memset(ones_col.bitcast(U32), ONE_BITS)
            eps_t = const.tile([1, 1], F32, tag="eps")
            nc.vector.memset(eps_t, EPS)
            seed_sb = const.tile([2, 3 * INNER_PC], F32R, tag="seed")
            nc.sync.dma_start(out=seed_sb, in_=seed[:, :])
            mask_sb = const.tile([KC, 4, QT], BF16, tag="mask")
            for m in range(4):
                nc.sync.dma_start(out=mask_sb[:, m, :], in_=masks[m, :, :])
            onmr = const.tile([2, tok], F32R, tag="onmr")  # row0=1, row1=-mu*rstd
            nc.vector.memset(onmr[0:1, :].bitcast(U32), ONE_BITS)
            # stats rows (heavily reused; SBUF rows cost 8KB/partition each)
            r0 = const.tile([1, tok], F32R, tag="r0")  # sums -> mu
            r1 = const.tile([1, tok], F32R, tag="r1")  # sumsq -> ex2 -> var -> rstd
            r2 = const.tile([1, tok], F32R, tag="r2")  # musq -> std -> nmr

            with (
                tc.tile_pool(name="xt", bufs=nkd) as xtp,
                tc.tile_pool(name="wqk", bufs=nkd) as wqkp,
                tc.tile_pool(name="psb", bufs=3) as ppool,
                tc.tile_pool(name="osb", bufs=2) as osbp,
                tc.tile_pool(name="rb", bufs=2) as rbp,
            ):
                # ---------------- phase A: load x^T, LN stats ----------------
                xt = []
                for kc in range(nkd):
                    t = xtp.tile([128, tok], F32R, tag="xt")
                    nc.sync.dma_start(out=t, in_=xT[kc * 128:(kc + 1) * 128, :])
                    xt.append(t)

                with (
                    tc.tile_pool(name="ps_stats", bufs=1, space="PSUM") as pstat,
                    tc.tile_pool(name="sqp", bufs=2) as sqp,
                ):
                    sum_ps = pstat.tile([1, tok], F32, tag="sum")
                    sq_ps = pstat.tile([1, tok], F32, tag="sq")
                    for kc in range(nkd):
                        for nt in range(nqt):
                            sl = slice(nt * QT, (nt + 1) * QT)
                            sq_t = sqp.tile([128, QT], F32R, tag="sq_t")
                            nc.vector.tensor_mul(sq_t, xt[kc][:, sl], xt[kc][:, sl])
                            nc.tensor.matmul(
                                out=sum_ps[0:1, sl], lhsT=_r(ones_col),
                                rhs=_r(xt[kc][:, sl]),
                                start=(kc == 0), stop=(kc == nkd - 1))
                            nc.tensor.matmul(
                                out=sq_ps[0:1, sl], lhsT=_r(ones_col),
                                rhs=_r(sq_t),
                                start=(kc == 0), stop=(kc == nkd - 1))
                    nc.vector.tensor_copy(r0, sum_ps)
                    nc.vector.tensor_copy(r1, sq_ps)

                # stats postprocessing on [1, tok] rows:
                # r0: sums -> mu (-> stays mu)
                # r1: sumsq -> ex2 -> rstd
                # r2: musq -> var -> std -> nmr
                nc.vector.tensor_scalar_mul(r0, r0, 1.0 / DIM)      # mu
                nc.vector.tensor_scalar_mul(r1, r1, 1.0 / DIM)      # ex2
                nc.vector.tensor_mul(r2, r0, r0)                    # mu^2
                nc.vector.tensor_sub(r2, r1, r2)                    # var
                nc.scalar.activation(out=r2, in_=r2, func=ACT_SQRT,
                                     bias=eps_t, scale=1.0)         # std
                nc.vector.reciprocal(r1, r2)                        # rstd
                nc.vector.scalar_tensor_tensor(
                    out=r2, in0=r0, scalar=-1.0, in1=r1,
                    op0=ALU.mult, op1=ALU.mult)                     # -mu*rstd
                nc.sync.dma_start(out=onmr[1:2, :], in_=r2[0:1, :])

                # xn^T = x^T * rstd (broadcast along partitions via K=1 matmul)
                with tc.tile_pool(name="ps_bc", bufs=nqt, space="PSUM") as pbc:
                    for nt in range(nqt):
                        sl = slice(nt * QT, (nt + 1) * QT)
                        bc = pbc.tile([128, QT], F32, tag="bc")
                        nc.tensor.matmul(out=bc, lhsT=_r(ones_row),
                                         rhs=_r(r1[0:1, sl]),
                                         start=True, stop=True)
                        for kc in range(nkd):
                            nc.vector.tensor_mul(xt[kc][:, sl], xt[kc][:, sl], bc)

                # ---------------- phase B-V: V (token-major) + ones column ----
                v_sb = []
                with (
                    tc.tile_pool(name="wv", bufs=nkd) as wvp,
                    tc.tile_pool(name="ps_v", bufs=2, space="PSUM") as psv,
                ):
                    wv = []
                    for kc in range(nkd):
                        t = wvp.tile([128, INNER_PC], F32R, tag="wv")
                        nc.sync.dma_start(
                            out=t, in_=w[kc * 128:(kc + 1) * 128,
                                         2 * INNER_PC:3 * INNER_PC])
                        wv.append(t)
                    for tt in range(ntt):
                        tsl = slice(tt * KC, (tt + 1) * KC)
                        v_ps = psv.tile([128, INNER_PC], F32, tag="v_ps")
                        nc.tensor.matmul(
                            out=v_ps, lhsT=_r(onmr[:, tsl]),
                            rhs=_r(seed_sb[:, 2 * INNER_PC:3 * INNER_PC]),
                            start=True, stop=False)
                        for kc in range(nkd):
                            nc.tensor.matmul(
                                out=v_ps, lhsT=_r(xt[kc][:, tsl]), rhs=_r(wv[kc]),
                                start=False, stop=(kc == nkd - 1))
                        vt = vpool.tile([128, HEADS_PC * (DH + 1)], BF16, tag="v_sb")
                        v3 = vt.rearrange("p (h w) -> p h w", w=DH + 1)
                        nc.vector.tensor_copy(
                            v3[:, :, 0:DH],
                            v_ps.rearrange("p (h w) -> p h w", w=DH))
                        nc.vector.memset(v3[:, :, DH:DH + 1], 1.0)
                        v_sb.append(vt)

                # ---------------- phases B-QK + C, per head pair --------------
                with (
                    tc.tile_pool(name="ps_qk", bufs=1, space="PSUM") as psqk,
                    tc.tile_pool(name="ps_s", bufs=2, space="PSUM") as pss,
                    tc.tile_pool(name="ps_o", bufs=2, space="PSUM") as pso,
                    tc.tile_pool(name="ps_rb", bufs=1, space="PSUM") as psrb,
                ):
                    for p in range(NPAIRS):
                        # -- QK projection for this pair (128 cols each of Q, K)
                        wqk = []
                        for kc in range(nkd):
                            t = wqkp.tile([128, 256], F32R, tag="wqk")
                            dsl = slice(kc * 128, (kc + 1) * 128)
                            nc.sync.dma_start(
                                out=t[:, 0:128],
                                in_=w[dsl, p * 128:(p + 1) * 128])
                            nc.sync.dma_start(
                                out=t[:, 128:256],
                                in_=w[dsl, INNER_PC + p * 128:INNER_PC + (p + 1) * 128])
                            wqk.append(t)
                        qT = qkp.tile([128, tok], F32R, tag="qT")
                        kT = qkp.tile([128, tok], F32R, tag="kT")
                        for dst, cofs, sofs in (
                            (qT, 0, p * 128),
                            (kT, 128, INNER_PC + p * 128),
                        ):
                            for nt in range(nqt):
                                sl = slice(nt * QT, (nt + 1) * QT)
                                ps = psqk.tile([128, QT], F32, tag="qk_ps")
                                nc.tensor.matmul(
                                    out=ps, lhsT=_r(seed_sb[:, sofs:sofs + 128]),
                                    rhs=_r(onmr[:, sl]), start=True, stop=False)
                                for kc in range(nkd):
                                    nc.tensor.matmul(
                                        out=ps,
                                        lhsT=_r(wqk[kc][:, cofs:cofs + 128]),
                                        rhs=_r(xt[kc][:, sl]),
                                        start=False, stop=(kc == nkd - 1))
                                nc.vector.tensor_copy(dst[:, sl], ps)

                        # -- causal attention for heads (2p, 2p+1)
                        for t_i in range(nqt):
                            qsl = slice(t_i * QT, (t_i + 1) * QT)
                            nch = (t_i + 1) * QT // KC
                            o_ps = [pso.tile([DH + 1, QT], F32, tag="o_ps", name=f"o_ps{h}")
                                    for h in range(2)]
                            for c in range(nch):
                                csl = slice(c * KC, (c + 1) * KC)
                                s_ps = pss.tile([128, 2 * QT], F32, tag="s_ps")
                                nc.tensor.matmul(
                                    out=s_ps[:, 0:QT],
                                    lhsT=_r(kT[0:DH, csl]), rhs=_r(qT[0:DH, qsl]),
                                    start=True, stop=True)
                                nc.tensor.matmul(
                                    out=s_ps[:, QT:2 * QT],
                                    lhsT=_r(kT[DH:128, csl]), rhs=_r(qT[DH:128, qsl]),
                                    start=True, stop=True)
                                p_sb = ppool.tile([128, 2 * QT], BF16, tag="p_sb")
                                nc.scalar.activation(out=p_sb, in_=s_ps,
                                                     func=ACT_EXP, scale=SCALE)
                                m = c - (nch - 4)
                                if m >= 0:
                                    nc.vector.tensor_mul(
                                        p_sb[:, 0:QT], p_sb[:, 0:QT], mask_sb[:, m, :])
                                    nc.vector.tensor_mul(
                                        p_sb[:, QT:2 * QT], p_sb[:, QT:2 * QT],
                                        mask_sb[:, m, :])
                                for h in range(2):
                                    hc = (2 * p + h) * (DH + 1)
                                    nc.tensor.matmul(
                                        out=o_ps[h],
                                        lhsT=v_sb[c][:, hc:hc + DH + 1],
                                        rhs=p_sb[:, h * QT:(h + 1) * QT],
                                        start=(c == 0), stop=(c == nch - 1))
                            for h in range(2):
                                recip = rbp.tile([1, QT], F32R, tag="recip")
                                nc.vector.reciprocal(recip, o_ps[h][DH:DH + 1, :])
                                rb_ps = psrb.tile([DH, QT], F32, tag="rb_ps")
                                nc.tensor.matmul(
                                    out=rb_ps, lhsT=ones_row[:, 0:DH], rhs=recip,
                                    start=True, stop=True)
                                rb = rbp.tile([DH, QT], F32, tag="rb")
                                nc.vector.tensor_copy(rb, rb_ps)
                                osb = osbp.tile([DH, QT], F32R, tag="osb")
                                nc.vector.scalar_tensor_tensor(
                                    out=osb, in0=o_ps[h][0:DH, :], scalar=1.0,
                                    in1=rb, op0=ALU.mult, op1=ALU.mult)
                                nc.sync.dma_start(
                                    out=oTs[p, h * DH:(h + 1) * DH, qsl], in_=osb)

            # ---------------- phase D: out projection ----------------
            with (
                tc.tile_pool(name="wo_sb", bufs=NPAIRS) as wop,
                tc.tile_pool(name="oL", bufs=2 * NPAIRS) as olp,
                tc.tile_pool(name="out_sb", bufs=3) as outp,
                tc.tile_pool(name="ps_out", bufs=2, space="PSUM") as psout,
            ):
                wos = []
                for p_i in range(NPAIRS):
                    t = wop.tile([128, DIM], F32R, tag="wo_sb")
                    nc.sync.dma_start(out=t, in_=wo[p_i * 128:(p_i + 1) * 128, :])
                    wos.append(t)
                for tt in range(ntt):
                    tsl = slice(tt * KC, (tt + 1) * KC)
                    ol = []
                    for p_i in range(NPAIRS):
                        t = olp.tile([128, KC], F32R, tag="oL")
                        nc.sync.dma_start(out=t, in_=oTs[p_i, :, tsl])
                        ol.append(t)
                    for nb in range(DIM // QT):
                        nsl = slice(nb * QT, (nb + 1) * QT)
                        ps = psout.tile([128, QT], F32, tag="out_ps")
                        for p_i in range(NPAIRS):
                            nc.tensor.matmul(
                                out=ps, lhsT=_r(ol[p_i]), rhs=_r(wos[p_i][:, nsl]),
                                start=(p_i == 0), stop=(p_i == NPAIRS - 1))
                        ob = outp.tile([128, QT], F32, tag="out_sb")
                        nc.vector.tensor_copy(ob, ps)
                        nc.sync.dma_start(out=out[tsl, nsl], in_=ob)

    return nc


def make_masks():
    import ml_dtypes

    j = np.arange(KC)[:, None]
    i = np.arange(QT)[None, :]
    return np.stack(
        [(i >= j + 128 * m) for m in range(4)]).astype(ml_dtypes.bfloat16)


def make_in_maps(x, ln_gamma, ln_beta, w_qkv, w_out):
    x = np.asarray(x, np.float32)
    g_ = np.asarray(ln_gamma, np.float32)
    b_ = np.asarray(ln_beta, np.float32)
    w_qkv = np.asarray(w_qkv, np.float32)
    w_out = np.asarray(w_out, np.float32)
    masks = make_masks()
    in_maps = []
    for c in range(8):
        b = c // 2
        g = c % 2
        cs = slice(g * INNER_PC, (g + 1) * INNER_PC)
        Wraw = np.concatenate(
            [w_qkv[:, 0 * DIM:1 * DIM][:, cs],
             w_qkv[:, 1 * DIM:2 * DIM][:, cs],
             w_qkv[:, 2 * DIM:3 * DIM][:, cs]], axis=1)
        Wp = (Wraw * g_[:, None]).astype(np.float32)
        seed = np.stack([b_ @ Wraw, Wp.sum(axis=0)]).astype(np.float32)
        in_maps.append({
            "xT": np.ascontiguousarray(x[b].T),
            "w": np.ascontiguousarray(Wp),
            "seed": seed,
            "wo": np.ascontiguousarray(w_out[cs, :]),
            "masks": masks,
        })
    return in_maps


_PROG = None


def kernel(x, ln_gamma, ln_beta, w_qkv, w_out):
    global _PROG
    from concourse.bass_utils import run_bass_kernel_spmd

    if _PROG is None:
        _PROG = build_program(TOK)
    in_maps = make_in_maps(x, ln_gamma, ln_beta, w_qkv, w_out)
    res = run_bass_kernel_spmd(_PROG, in_maps, list(range(8)))
    parts = [res.results[c]["out"] for c in range(8)]
    out = np.empty((B, TOK, DIM), np.float32)
    for b in range(B):
        out[b] = parts[2 * b] + parts[2 * b + 1]
    return out



# revision 22
# speedup vs baseline: 1.6551x; 1.6551x over previous
"""Causal attention block (LN -> QKV -> causal MHA -> out-proj) on 8 trn2
NeuronCores via Bass/Tile.

Sharding: core c handles batch b=c//2 and head-group g=c%2 (8 of 16 heads).
Data parallel over batch, tensor parallel over heads; the out-proj partial
sums (2 per batch) are reduced on the host during the gather, so the device
program needs no collectives and is pure SPMD.

v2 design (bf16 everywhere, PE kept warm, no DRAM roundtrips):
  - all matmul operands bf16 (full-rate 1 cyc/row, FWL weight loads, half
    the DMA); PSUM accumulation stays fp32.
  - LN stats land in a [4, 512] PSUM layout so the postprocessing runs on
    4 partitions instead of 1; rstd is folded into the PSUM->SBUF copies
    (per-partition tensor_scalar for V, a broadcast-row multiply for Q/K),
    so x itself is never rewritten.  The mean/beta correction enters each
    projection as a K=2 seed matmul with rows [std, -mu].
  - scores are computed transposed per 128-key chunk with 2 heads packed;
    exp runs on ACT straight out of PSUM with the 1/sqrt(dh) scale folded
    in; diagonal chunks are narrowed to the valid q-range (saves PE + ACT)
    and only the 128x128 boundary block gets a 0/1 mask multiply.
  - softmax denominators ride as a 65th ones-column of V through the P@V
    matmul; per pair they are DMA-gathered into a [64, 64] tile for one
    batched reciprocal, spread back with two small DMAs, and applied in the
    normalize copy into the SBUF-resident O^T (no DRAM scratch).
  - the QK projection of pair p+1 is interleaved into the (ACT-bound)
    attention stream of pair p so the PE never idles long enough to be
    clock-throttled.
"""

import numpy as np

import concourse.bass as bass
import concourse.mybir as mybir
import concourse.tile as tile_mod

# ----------------------------------------------------------------------------
# Workaround for this walrus build rejecting instructions that carry more than
# MAX_WAITS semaphore waits ("Too many sync wait commands" in CoreV3GenImpl
# setupSyncWait — hit on Drain and Matmult/S3_LW encodings). Split excess
# waits onto single-wait NOP carrier instructions emitted just before the
# original instruction on the same engine: program order on the sequencer
# makes this semantically identical (waits are AND conditions).
# ----------------------------------------------------------------------------
_MAX_WAITS = 1
_orig_add_instruction = tile_mod.TileContext._add_instruction
_carrier_id = [0]


def _split_waits_add_instruction(self, inst):
    si = inst.sync_info
    if (
        si is not None
        and si.on_wait
        and len(si.on_wait) > _MAX_WAITS
        and inst.engine != mybir.EngineType.Unassigned
    ):
        waits = list(si.on_wait)
        keep = waits[-_MAX_WAITS:]
        for w in waits[:-_MAX_WAITS]:
            _carrier_id[0] += 1
            nop = mybir.InstNoOp(name=f"I-waitc-{_carrier_id[0]}")
            nop.engine = inst.engine
            nop.sync_info = mybir.SyncInfo(on_wait=[w], on_update=[])
            _orig_add_instruction(self, nop)
        inst.sync_info = mybir.SyncInfo(
            on_wait=keep,
            on_update=list(si.on_update) if si.on_update else [],
        )
    _orig_add_instruction(self, inst)


tile_mod.TileContext._add_instruction = _split_waits_add_instruction

from concourse.vector_clock import ScopedClock


def _patched_drain_and_barrier(self, tick_clock, wait_clock):
    # Same wait-splitting for the TileContext exit drain, which is emitted
    # after lowering (outside _add_instruction).
    nc = self.nc
    carrier = nc.sync.nop(nofuse=True)
    wait_clock.add_sem_waits(carrier.ins, ScopedClock({None: tick_clock.global_clock}))
    si = carrier.ins.sync_info
    waits = list(si.on_wait) if si is not None and si.on_wait else []
    if len(waits) > _MAX_WAITS:
        carrier.ins.sync_info = mybir.SyncInfo(
            on_wait=waits[:_MAX_WAITS],
            on_update=list(si.on_update) if si.on_update else [],
        )
        rest = waits[_MAX_WAITS:]
        while rest:
            extra = nc.sync.nop(nofuse=True)
            extra.ins.sync_info = mybir.SyncInfo(
                on_wait=rest[:_MAX_WAITS], on_update=[])
            rest = rest[_MAX_WAITS:]

    nc.sync.drain()
    nc.all_engine_barrier()
    assert self.sems is not None
    popped = nc._tile_sem_poison_stack.pop()
    assert popped is self._sem_poison
    nc.clear_and_free_semaphores(list(self.sems.allocated().values()))
    nc.all_engine_barrier()


tile_mod.TileContext._drain_and_barrier = _patched_drain_and_barrier

# ----------------------------------------------------------------------------

F32 = mybir.dt.float32
BF16 = mybir.dt.bfloat16
ALU = mybir.AluOpType
ACT_EXP = mybir.ActivationFunctionType.Exp
ACT_SQRT = mybir.ActivationFunctionType.Sqrt

B = 4
TOK = 2048
DIM = 1024
HEADS = 16
DH = 64
HEADS_PC = 8          # heads per core
INNER_PC = HEADS_PC * DH  # 512
NPAIRS = HEADS_PC // 2
QT = 512              # query tile
KC = 128              # key-token chunk
NQT = TOK // QT       # 4
NTT = TOK // KC       # 16
NKD = DIM // 128      # 8
EPS = 1e-5
SCALE = DH ** -0.5


def _ap0(ap, parts):
    """Partition-broadcast AP (stride-0 leading dim) for DMA reads."""
    return bass.AP(tensor=ap.tensor, offset=ap.offset, ap=[[0, parts]] + list(ap.ap))


def _pstride(t, step, n):
    """View tile t's partitions with a stride (partition p -> p*step)."""
    return bass.AP(tensor=t.tensor, offset=t.offset,
                   ap=[[t.ap[0][0] * step, n]] + list(t.ap[1:]))


def build_program(tok=TOK):
    nc = bass.Bass()
    xT = nc.declare_dram_parameter("xT", [DIM, tok], BF16, isOutput=False)
    w = nc.declare_dram_parameter("w", [DIM, 3 * INNER_PC], BF16, isOutput=False)
    seed = nc.declare_dram_parameter("seed", [2, 3 * INNER_PC], BF16, isOutput=False)
    wo = nc.declare_dram_parameter("wo", [INNER_PC, DIM], BF16, isOutput=False)
    mask_d = nc.declare_dram_parameter("mask128", [KC, KC], BF16, isOutput=False)
    eye_d = nc.declare_dram_parameter("eye4", [4, 4], F32, isOutput=False)
    out = nc.declare_dram_parameter("out", [tok, DIM], F32, isOutput=True)
    # DRAM bounce buffers for partition-broadcasts (stride-0 reads are only
    # legal on the DRAM side of a DMA)
    rstd_d = nc.dram_tensor("rstd_row", [1, tok], F32)
    rec_d = nc.dram_tensor("rec_row", [NPAIRS, 8 * QT], BF16)

    with tile_mod.TileContext(nc) as tc, nc.allow_low_precision(
            "bf16 operand tiles; all matmul accumulation stays fp32 PSUM"):
        with (
            tc.tile_pool(name="const", bufs=1) as const,
            tc.tile_pool(name="xt", bufs=NKD) as xtp,
            tc.tile_pool(name="wsb", bufs=NKD) as wp,
            tc.tile_pool(name="wosb", bufs=NPAIRS) as wop,
            tc.tile_pool(name="vpool", bufs=NTT) as vpool,
            tc.tile_pool(name="qkT", bufs=2) as qkp,
            tc.tile_pool(name="osb", bufs=2) as osbp,
            tc.tile_pool(name="oT", bufs=NPAIRS) as oTp,
            tc.tile_pool(name="psb", bufs=3) as ppool,
            tc.tile_pool(name="den", bufs=1) as denp,
            tc.tile_pool(name="sq", bufs=2) as sqp,
        ):
            # ---------------- constants / big loads ----------------
            ones_col = const.tile([128, 1], BF16, tag="ones_col")
            nc.vector.memset(ones_col, 1.0)
            # sel4: 4 stationary variants [128, 4]; variant v is all-ones in
            # column v, zero elsewhere.  Routes LN stat rows to consecutive
            # PSUM partitions (matmul/DVE PSUM bases must be 32-aligned).
            sel4 = const.tile([128, 16], BF16, tag="sel4")
            nc.vector.memset(sel4, 0.0)
            for v in range(4):
                nc.vector.memset(sel4[:, 5 * v:5 * v + 1], 1.0)
            eps4 = const.tile([4, 1], F32, tag="eps")
            nc.vector.memset(eps4, EPS)
            mask_sb = const.tile([KC, KC], BF16, tag="mask")
            nc.sync.dma_start(out=mask_sb, in_=mask_d[:, :])
            eye4 = const.tile([4, 4], F32, tag="eye4")
            nc.sync.dma_start(out=eye4, in_=eye_d[:, :])
            seed_sb = const.tile([2, 3 * INNER_PC], BF16, tag="seed")
            nc.sync.dma_start(out=seed_sb, in_=seed[:, :])

            xt = []
            for kc in range(NKD):
                t = xtp.tile([128, tok], BF16, tag="xt")
                nc.sync.dma_start(out=t, in_=xT[kc * 128:(kc + 1) * 128, :])
                xt.append(t)
            wsb = []
            for kc in range(NKD):
                t = wp.tile([128, 3 * INNER_PC], BF16, tag="wsb")
                nc.sync.dma_start(out=t, in_=w[kc * 128:(kc + 1) * 128, :])
                wsb.append(t)
            wos = []
            for p in range(NPAIRS):
                t = wop.tile([128, DIM], BF16, tag="wosb")
                nc.sync.dma_start(out=t, in_=wo[p * 128:(p + 1) * 128, :])
                wos.append(t)

            # LN-derived rows (device computed, long-lived)
            onm = const.tile([2, tok], BF16, tag="onm")        # r0=std r1=-mu
            bc_sb = const.tile([128, tok], F32, tag="bc")      # rstd bcast
            # NOTE: rstd_col columns are block-permuted: chunk tt lives at
            # column 4*(tt%4) + tt//4 (transpose blocks land contiguously).
            rstd_col = const.tile([128, NTT], F32, tag="rstdc")
            rstd4 = const.tile([4, QT], F32, tag="rstd4")
            std4 = const.tile([4, QT], F32, tag="std4")
            std4b = const.tile([4, QT], BF16, tag="std4b")
            nmu4b = const.tile([4, QT], BF16, tag="nmu4b")
            mu4 = const.tile([4, QT], F32, tag="mu4")
            var4 = const.tile([4, QT], F32, tag="var4")
            musq4 = const.tile([4, QT], F32, tag="musq4")

            # ---------------- phase A: LN stats ----------------
            with (
                tc.tile_pool(name="ps_stats", bufs=1, space="PSUM") as pstat,
                tc.tile_pool(name="ps_tr", bufs=1, space="PSUM") as ptr,
            ):
                # Two [4, 512] PSUM tiles: token-slice nt's sum / sum-of-sq
                # rows land on partition nt via the sel4 stationary (other
                # rows accumulate zero), one accumulation group per tile.
                sum_ps = pstat.tile([4, QT], F32, tag="sum")
                sq_ps = pstat.tile([4, QT], F32, tag="sq")
                for kc in range(NKD):
                    for nt in range(NQT):
                        sl = slice(nt * QT, (nt + 1) * QT)
                        first = kc == 0 and nt == 0
                        last = kc == NKD - 1 and nt == NQT - 1
                        sq_t = sqp.tile([128, QT], BF16, tag="sq_t")
                        nc.vector.tensor_mul(sq_t, xt[kc][:, sl], xt[kc][:, sl])
                        nc.tensor.matmul(
                            out=sum_ps, lhsT=sel4[:, 4 * nt:4 * nt + 4],
                            rhs=xt[kc][:, sl],
                            start=first, stop=last)
                        nc.tensor.matmul(
                            out=sq_ps, lhsT=sel4[:, 4 * nt:4 * nt + 4],
                            rhs=sq_t,
                            start=first, stop=last)
                # postproc on [4, 512] (4 lanes)
                nc.vector.tensor_scalar_mul(mu4, sum_ps, 1.0 / DIM)
                nc.vector.tensor_scalar_mul(var4, sq_ps, 1.0 / DIM)
                nc.vector.tensor_mul(musq4, mu4, mu4)
                nc.vector.tensor_sub(var4, var4, musq4)
                nc.scalar.activation(out=std4, in_=var4, func=ACT_SQRT,
                                     bias=eps4, scale=1.0)
                nc.vector.reciprocal(rstd4, std4)
                nc.vector.tensor_copy(std4b, std4)
                nc.vector.tensor_scalar_mul(nmu4b, mu4, -1.0)
                # gather LN rows into operand layouts (cross-shape DMAs:
                # only total element count must match)
                nc.sync.dma_start(out=onm[0:1, :], in_=std4b[:, :])
                nc.sync.dma_start(out=onm[1:2, :], in_=nmu4b[:, :])
                # rstd broadcast [128, tok]: bounce through DRAM, then a
                # stride-0 partition-broadcast read
                nc.sync.dma_start(out=rstd_d[0:1, :], in_=rstd4[:, :])
                nc.sync.dma_start(
                    out=bc_sb,
                    in_=bass.AP(tensor=rstd_d, offset=0,
                                ap=[[0, 128], [1, tok]]))
                # rstd_col via 4 PE transposes of [4, 128] blocks; block g
                # lands at contiguous cols [4g, 4g+4) giving the permuted
                # column order documented above.
                rc_ps = ptr.tile([128, NTT], F32, tag="rc")
                for g in range(4):
                    nc.tensor.transpose(
                        out=rc_ps[:, 4 * g:4 * (g + 1)],
                        in_=rstd4[:, g * 128:(g + 1) * 128],
                        identity=eye4)
                nc.vector.tensor_copy(rstd_col, rc_ps)

            # ---------------- phase B: V projection ----------------
            v_sb = []
            with tc.tile_pool(name="ps_v", bufs=2, space="PSUM") as psv:
                for tt in range(NTT):
                    tsl = slice(tt * KC, (tt + 1) * KC)
                    v_ps = psv.tile([128, INNER_PC], F32, tag="v_ps")
                    nc.tensor.matmul(
                        out=v_ps, lhsT=onm[:, tsl],
                        rhs=seed_sb[:, 2 * INNER_PC:3 * INNER_PC],
                        start=True, stop=False)
                    for kc in range(NKD):
                        nc.tensor.matmul(
                            out=v_ps, lhsT=xt[kc][:, tsl],
                            rhs=wsb[kc][:, 2 * INNER_PC:3 * INNER_PC],
                            start=False, stop=(kc == NKD - 1))
                    vt = vpool.tile([128, HEADS_PC * (DH + 1)], BF16, tag="v_sb")
                    v3 = vt.rearrange("p (h w) -> p h w", w=DH + 1)
                    pc = 4 * (tt % 4) + tt // 4  # permuted rstd_col index
                    nc.vector.tensor_scalar(
                        out=v3[:, :, 0:DH],
                        in0=v_ps.rearrange("p (h w) -> p h w", w=DH),
                        scalar1=rstd_col[:, pc:pc + 1], scalar2=None,
                        op0=ALU.mult)
                    nc.vector.memset(v3[:, :, DH:DH + 1], 1.0)
                    v_sb.append(vt)

            # ---------------- phases C: QK proj + attention ----------------
            with (
                tc.tile_pool(name="ps_qk", bufs=1, space="PSUM") as psqk,
                tc.tile_pool(name="ps_s", bufs=2, space="PSUM") as pss,
                tc.tile_pool(name="ps_o", bufs=3, space="PSUM") as pso,
            ):
                def emit_qk_slice(p, dst, d, nt):
                    """One [128, 512] token-slice of the Q or K projection of
                    pair p (d=0 -> Q, d=1 -> K)."""
                    sl = slice(nt * QT, (nt + 1) * QT)
                    cofs = d * INNER_PC + p * 128
                    ps = psqk.tile([128, QT], F32, tag="qk_ps")
                    nc.tensor.matmul(
                        out=ps, lhsT=seed_sb[:, cofs:cofs + 128],
                        rhs=onm[:, sl], start=True, stop=False)
                    for kc in range(NKD):
                        nc.tensor.matmul(
                            out=ps, lhsT=wsb[kc][:, cofs:cofs + 128],
                            rhs=xt[kc][:, sl],
                            start=False, stop=(kc == NKD - 1))
                    nc.vector.tensor_mul(dst[:, sl], ps, bc_sb[:, sl])

                def emit_qk_pair(p):
                    qT = qkp.tile([128, tok], BF16, tag="qT")
                    kT = qkp.tile([128, tok], BF16, tag="kT")
                    for d, dst in ((0, qT), (1, kT)):
                        for nt in range(NQT):
                            emit_qk_slice(p, dst, d, nt)
                    return qT, kT

                qT, kT = emit_qk_pair(0)
                oTs = []
                for p in range(NPAIRS):
                    # list of deferred QK slices of pair p+1 to interleave
                    nxt = []
                    if p + 1 < NPAIRS:
                        qT2 = qkp.tile([128, tok], BF16, tag="qT")
                        kT2 = qkp.tile([128, tok], BF16, tag="kT")
                        nxt = [(p + 1, dst, d, nt)
                               for d, dst in ((0, qT2), (1, kT2))
                               for nt in range(NQT)]

                    o_sb = [osbp.tile([DH + 1, tok], BF16, tag=f"o_sb{h}",
                                      name=f"o_sb{p}_{h}")
                            for h in range(2)]
                    for t_i in range(NQT):
                        qsl0 = t_i * QT
                        nch = (t_i + 1) * QT // KC
                        o_ps = [pso.tile([DH + 1, QT], F32, tag="o_ps",
                                         name=f"o_ps{p}_{t_i}_{h}")
                                for h in range(2)]
                        p_tiles = {}

                        def emit_scores(c):
                            m = c - (nch - 4)
                            lo = 128 * m if m > 0 else 0
                            csl = slice(c * KC, (c + 1) * KC)
                            s_ps = pss.tile([128, 2 * QT], F32, tag="s_ps")
                            p_sb = ppool.tile([128, 2 * QT], BF16, tag="p_sb")
                            for h in range(2):
                                nc.tensor.matmul(
                                    out=s_ps[:, h * QT + lo:(h + 1) * QT],
                                    lhsT=kT[h * DH:(h + 1) * DH, csl],
                                    rhs=qT[h * DH:(h + 1) * DH,
                                           qsl0 + lo:qsl0 + QT],
                                    start=True, stop=True)
                            s3 = s_ps.rearrange("p (h q) -> p h q", q=QT)
                            p3 = p_sb.rearrange("p (h q) -> p h q", q=QT)
                            nc.scalar.activation(
                                out=p3[:, :, lo:QT], in_=s3[:, :, lo:QT],
                                func=ACT_EXP, scale=SCALE)
                            if m >= 0:
                                for h in range(2):
                                    nc.vector.tensor_mul(
                                        p_sb[:, h * QT + lo:h * QT + lo + KC],
                                        p_sb[:, h * QT + lo:h * QT + lo + KC],
                                        mask_sb)
                            p_tiles[c] = p_sb

                        def emit_pv(c):
                            m = c - (nch - 4)
                            lo = 128 * m if m > 0 else 0
                            p_sb = p_tiles.pop(c)
                            for h in range(2):
                                hc = (2 * p + h) * (DH + 1)
                                nc.tensor.matmul(
                                    out=o_ps[h][:, lo:QT],
                                    lhsT=v_sb[c][:, hc:hc + DH + 1],
                                    rhs=p_sb[:, h * QT + lo:(h + 1) * QT],
                                    start=(c == 0), stop=(c == nch - 1),
                                    skip_group_check=True)

                        emit_scores(0)
                        for c in range(1, nch):
                            emit_scores(c)
                            emit_pv(c - 1)
                        emit_pv(nch - 1)
                        # free PSUM fast: copy [65, 512] (O^T rows + denom row)
                        for h in range(2):
                            nc.vector.tensor_copy(
                                o_sb[h][:, qsl0:qsl0 + QT], o_ps[h])
                        # interleave two QK slices of the next pair
                        for _ in range(2):
                            if nxt:
                                emit_qk_slice(*nxt.pop(0))
                    while nxt:
                        emit_qk_slice(*nxt.pop(0))

                    # ---- softmax denominators -> reciprocal -> normalize
                    # den64 partition 8*(2t+h)+j holds tokens [64j, 64j+64)
                    # of (qtile t, head h); flattening partition-major gives
                    # rec_row offsets 512*(2t+h)+64j+e, i.e. (t, h)-blocked.
                    den64 = denp.tile([64, 64], BF16, tag="den64")
                    rec64 = denp.tile([64, 64], BF16, tag="rec64")
                    rb = denp.tile([64, 8 * QT], BF16, tag="rb")
                    for t_i in range(NQT):
                        for h in range(2):
                            r0 = 8 * (2 * t_i + h)
                            nc.sync.dma_start(
                                out=den64[r0:r0 + 8, :],
                                in_=o_sb[h][DH:DH + 1,
                                            t_i * QT:(t_i + 1) * QT])
                    nc.vector.reciprocal(rec64, den64)
                    nc.sync.dma_start(out=rec_d[p:p + 1, :], in_=rec64[:, :])
                    nc.sync.dma_start(
                        out=rb,
                        in_=bass.AP(tensor=rec_d, offset=p * 8 * QT,
                                    ap=[[0, 64], [1, 8 * QT]]))
                    oT = oTp.tile([128, tok], BF16, tag="oT", name=f"oT{p}")
                    for t_i in range(NQT):
                        qsl = slice(t_i * QT, (t_i + 1) * QT)
                        for h in range(2):
                            rsl = slice((2 * t_i + h) * QT,
                                        (2 * t_i + h + 1) * QT)
                            nc.vector.scalar_tensor_tensor(
                                out=oT[h * DH:(h + 1) * DH, qsl],
                                in0=o_sb[h][0:DH, qsl], scalar=1.0,
                                in1=rb[:, rsl],
                                op0=ALU.mult, op1=ALU.mult)
                    oTs.append(oT)
                    if p + 1 < NPAIRS:
                        qT, kT = qT2, kT2

            # ---------------- phase D: out projection ----------------
            with (
                tc.tile_pool(name="ps_out", bufs=2, space="PSUM") as psout,
                tc.tile_pool(name="out_sb", bufs=2) as outp,
            ):
                for tt in range(NTT):
                    tsl = slice(tt * KC, (tt + 1) * KC)
                    for nb in range(DIM // QT):
                        nsl = slice(nb * QT, (nb + 1) * QT)
                        ps = psout.tile([128, QT], F32, tag="out_ps")
                        for p in range(NPAIRS):
                            nc.tensor.matmul(
                                out=ps, lhsT=oTs[p][:, tsl],
                                rhs=wos[p][:, nsl],
                                start=(p == 0), stop=(p == NPAIRS - 1))
                        ob = outp.tile([128, QT], F32, tag="out_sb")
                        nc.vector.tensor_copy(ob, ps)
                        nc.sync.dma_start(out=out[tsl, nsl], in_=ob)

    return nc


def make_masks():
    import ml_dtypes

    k = np.arange(KC)[:, None]
    q = np.arange(KC)[None, :]
    return (q >= k).astype(ml_dtypes.bfloat16)


def make_in_maps(x, ln_gamma, ln_beta, w_qkv, w_out):
    import ml_dtypes

    bf16 = ml_dtypes.bfloat16
    x = np.asarray(x, np.float32)
    g_ = np.asarray(ln_gamma, np.float32)
    b_ = np.asarray(ln_beta, np.float32)
    w_qkv = np.asarray(w_qkv, np.float32)
    w_out = np.asarray(w_out, np.float32)
    mask128 = make_masks()
    eye4 = np.eye(4, dtype=np.float32)
    in_maps = []
    for c in range(8):
        b = c // 2
        g = c % 2
        cs = slice(g * INNER_PC, (g + 1) * INNER_PC)
        Wraw = np.concatenate(
            [w_qkv[:, 0 * DIM:1 * DIM][:, cs],
             w_qkv[:, 1 * DIM:2 * DIM][:, cs],
             w_qkv[:, 2 * DIM:3 * DIM][:, cs]], axis=1)
        Wp = (Wraw * g_[:, None]).astype(bf16)
        seed = np.stack([b_ @ Wraw,
                         Wp.astype(np.float32).sum(axis=0)]).astype(bf16)
        in_maps.append({
            "xT": np.ascontiguousarray(x[b].T).astype(bf16),
            "w": np.ascontiguousarray(Wp),
            "seed": seed,
            "wo": np.ascontiguousarray(w_out[cs, :]).astype(bf16),
            "mask128": mask128,
            "eye4": eye4,
        })
    return in_maps


_PROG = None


def kernel(x, ln_gamma, ln_beta, w_qkv, w_out):
    global _PROG
    from concourse.bass_utils import run_bass_kernel_spmd

    if _PROG is None:
        _PROG = build_program(TOK)
    in_maps = make_in_maps(x, ln_gamma, ln_beta, w_qkv, w_out)
    res = run_bass_kernel_spmd(_PROG, in_maps, list(range(8)))
    parts = [res.results[c]["out"] for c in range(8)]
    out = np.empty((B, TOK, DIM), np.float32)
    for b in range(B):
        out[b] = parts[2 * b] + parts[2 * b + 1]
    return out


# revision 28
# speedup vs baseline: 1.8219x; 1.1008x over previous
"""Causal attention block (LN -> QKV -> causal MHA -> out-proj) on 8 trn2
NeuronCores via Bass/Tile.

Sharding: core c handles batch b=c//2 and head-group g=c%2 (8 of 16 heads).
Data parallel over batch, tensor parallel over heads; the out-proj partial
sums (2 per batch) are reduced on the host during the gather, so the device
program needs no collectives and is pure SPMD.

v2 design (bf16 everywhere, PE kept warm, no DRAM roundtrips):
  - all matmul operands bf16 (full-rate 1 cyc/row, FWL weight loads, half
    the DMA); PSUM accumulation stays fp32.
  - LN stats land in a [4, 512] PSUM layout so the postprocessing runs on
    4 partitions instead of 1; rstd is folded into the PSUM->SBUF copies
    (per-partition tensor_scalar for V, a broadcast-row multiply for Q/K),
    so x itself is never rewritten.  The mean/beta correction enters each
    projection as a K=2 seed matmul with rows [std, -mu].
  - scores are computed transposed per 128-key chunk with 2 heads packed;
    exp runs on ACT straight out of PSUM with the 1/sqrt(dh) scale folded
    in; diagonal chunks are narrowed to the valid q-range (saves PE + ACT)
    and only the 128x128 boundary block gets a 0/1 mask multiply.
  - softmax denominators ride as a 65th ones-column of V through the P@V
    matmul; per pair they are DMA-gathered into a [64, 64] tile for one
    batched reciprocal, spread back with two small DMAs, and applied in the
    normalize copy into the SBUF-resident O^T (no DRAM scratch).
  - the QK projection of pair p+1 is interleaved into the (ACT-bound)
    attention stream of pair p so the PE never idles long enough to be
    clock-throttled.
"""

import numpy as np

import concourse.bass as bass
import concourse.mybir as mybir
import concourse.tile as tile_mod

# ----------------------------------------------------------------------------
# Workaround for this walrus build rejecting instructions that carry more than
# MAX_WAITS semaphore waits ("Too many sync wait commands" in CoreV3GenImpl
# setupSyncWait — hit on Drain and Matmult/S3_LW encodings). Split excess
# waits onto single-wait NOP carrier instructions emitted just before the
# original instruction on the same engine: program order on the sequencer
# makes this semantically identical (waits are AND conditions).
# ----------------------------------------------------------------------------
_MAX_WAITS = 1
_orig_add_instruction = tile_mod.TileContext._add_instruction
_carrier_id = [0]


def _split_waits_add_instruction(self, inst):
    si = inst.sync_info
    if (
        si is not None
        and si.on_wait
        and len(si.on_wait) > _MAX_WAITS
        and inst.engine != mybir.EngineType.Unassigned
    ):
        waits = list(si.on_wait)
        keep = waits[-_MAX_WAITS:]
        for w in waits[:-_MAX_WAITS]:
            _carrier_id[0] += 1
            nop = mybir.InstNoOp(name=f"I-waitc-{_carrier_id[0]}")
            nop.engine = inst.engine
            nop.sync_info = mybir.SyncInfo(on_wait=[w], on_update=[])
            _orig_add_instruction(self, nop)
        inst.sync_info = mybir.SyncInfo(
            on_wait=keep,
            on_update=list(si.on_update) if si.on_update else [],
        )
    _orig_add_instruction(self, inst)


tile_mod.TileContext._add_instruction = _split_waits_add_instruction

from concourse.vector_clock import ScopedClock


def _patched_drain_and_barrier(self, tick_clock, wait_clock):
    # Same wait-splitting for the TileContext exit drain, which is emitted
    # after lowering (outside _add_instruction).
    nc = self.nc
    carrier = nc.sync.nop(nofuse=True)
    wait_clock.add_sem_waits(carrier.ins, ScopedClock({None: tick_clock.global_clock}))
    si = carrier.ins.sync_info
    waits = list(si.on_wait) if si is not None and si.on_wait else []
    if len(waits) > _MAX_WAITS:
        carrier.ins.sync_info = mybir.SyncInfo(
            on_wait=waits[:_MAX_WAITS],
            on_update=list(si.on_update) if si.on_update else [],
        )
        rest = waits[_MAX_WAITS:]
        while rest:
            extra = nc.sync.nop(nofuse=True)
            extra.ins.sync_info = mybir.SyncInfo(
                on_wait=rest[:_MAX_WAITS], on_update=[])
            rest = rest[_MAX_WAITS:]

    nc.sync.drain()
    nc.all_engine_barrier()
    assert self.sems is not None
    popped = nc._tile_sem_poison_stack.pop()
    assert popped is self._sem_poison
    nc.clear_and_free_semaphores(list(self.sems.allocated().values()))
    nc.all_engine_barrier()


tile_mod.TileContext._drain_and_barrier = _patched_drain_and_barrier

# ----------------------------------------------------------------------------

F32 = mybir.dt.float32
BF16 = mybir.dt.bfloat16
ALU = mybir.AluOpType
ACT_EXP = mybir.ActivationFunctionType.Exp
ACT_SQRT = mybir.ActivationFunctionType.Sqrt

B = 4
TOK = 2048
DIM = 1024
HEADS = 16
DH = 64
HEADS_PC = 8          # heads per core
INNER_PC = HEADS_PC * DH  # 512
NPAIRS = HEADS_PC // 2
QT = 512              # query tile
KC = 128              # key-token chunk
NQT = TOK // QT       # 4
NTT = TOK // KC       # 16
NKD = DIM // 128      # 8
EPS = 1e-5
SCALE = DH ** -0.5


def _ap0(ap, parts):
    """Partition-broadcast AP (stride-0 leading dim) for DMA reads."""
    return bass.AP(tensor=ap.tensor, offset=ap.offset, ap=[[0, parts]] + list(ap.ap))


def _pstride(t, step, n):
    """View tile t's partitions with a stride (partition p -> p*step)."""
    return bass.AP(tensor=t.tensor, offset=t.offset,
                   ap=[[t.ap[0][0] * step, n]] + list(t.ap[1:]))


def build_program(tok=TOK):
    nc = bass.Bass()
    xT = nc.declare_dram_parameter("xT", [DIM, tok], BF16, isOutput=False)
    w = nc.declare_dram_parameter("w", [DIM, 3 * INNER_PC], BF16, isOutput=False)
    seed = nc.declare_dram_parameter("seed", [2, 3 * INNER_PC], BF16, isOutput=False)
    wo = nc.declare_dram_parameter("wo", [INNER_PC, DIM], BF16, isOutput=False)
    mask_d = nc.declare_dram_parameter("mask128", [KC, KC], BF16, isOutput=False)
    eye_d = nc.declare_dram_parameter("eye4", [4, 4], F32, isOutput=False)
    out = nc.declare_dram_parameter("out", [tok, DIM], F32, isOutput=True)
    # DRAM bounce buffers for partition-broadcasts (stride-0 reads are only
    # legal on the DRAM side of a DMA)
    rstd_d = nc.dram_tensor("rstd_row", [1, tok], F32)
    rec_d = nc.dram_tensor("rec_row", [NPAIRS, 8 * QT], BF16)

    with tile_mod.TileContext(nc) as tc, nc.allow_low_precision(
            "bf16 operand tiles; all matmul accumulation stays fp32 PSUM"):
        with (
            tc.tile_pool(name="const", bufs=1) as const,
            tc.tile_pool(name="xt", bufs=NKD) as xtp,
            tc.tile_pool(name="wsb", bufs=NKD) as wp,
            tc.tile_pool(name="wosb", bufs=NPAIRS) as wop,
            tc.tile_pool(name="vpool", bufs=NTT) as vpool,
            tc.tile_pool(name="qkT", bufs=2) as qkp,
            tc.tile_pool(name="osb", bufs=3) as osbp,
            tc.tile_pool(name="oT", bufs=NPAIRS) as oTp,
            tc.tile_pool(name="psb", bufs=3) as ppool,
            tc.tile_pool(name="den", bufs=2) as denp,
            tc.tile_pool(name="sq", bufs=2) as sqp,
        ):
            # ---------------- constants / big loads ----------------
            ones_col = const.tile([128, 1], BF16, tag="ones_col")
            nc.vector.memset(ones_col, 1.0)
            # sel4: 4 stationary variants [128, 4]; variant v is all-ones in
            # column v, zero elsewhere.  Routes LN stat rows to consecutive
            # PSUM partitions (matmul/DVE PSUM bases must be 32-aligned).
            sel4 = const.tile([128, 16], BF16, tag="sel4")
            nc.vector.memset(sel4, 0.0)
            for v in range(4):
                nc.vector.memset(sel4[:, 5 * v:5 * v + 1], 1.0)
            eps4 = const.tile([4, 1], F32, tag="eps")
            nc.vector.memset(eps4, EPS)
            mask_sb = const.tile([KC, KC], BF16, tag="mask")
            nc.sync.dma_start(out=mask_sb, in_=mask_d[:, :])
            eye4 = const.tile([4, 4], F32, tag="eye4")
            nc.sync.dma_start(out=eye4, in_=eye_d[:, :])
            seed_sb = const.tile([2, 3 * INNER_PC], BF16, tag="seed")
            nc.sync.dma_start(out=seed_sb, in_=seed[:, :])

            xt = []
            for kc in range(NKD):
                t = xtp.tile([128, tok], BF16, tag="xt")
                nc.sync.dma_start(out=t, in_=xT[kc * 128:(kc + 1) * 128, :])
                xt.append(t)
            wsb = []
            for kc in range(NKD):
                t = wp.tile([128, 3 * INNER_PC], BF16, tag="wsb")
                nc.sync.dma_start(out=t, in_=w[kc * 128:(kc + 1) * 128, :])
                wsb.append(t)
            wos = []
            for p in range(NPAIRS):
                t = wop.tile([128, DIM], BF16, tag="wosb")
                nc.sync.dma_start(out=t, in_=wo[p * 128:(p + 1) * 128, :])
                wos.append(t)

            # LN-derived rows (device computed, long-lived)
            onm = const.tile([2, tok], BF16, tag="onm")        # r0=std r1=-mu
            bc_sb = const.tile([128, tok], F32, tag="bc")      # rstd bcast
            # NOTE: rstd_col columns are block-permuted: chunk tt lives at
            # column 4*(tt%4) + tt//4 (transpose blocks land contiguously).
            rstd_col = const.tile([128, NTT], F32, tag="rstdc")
            rstd4 = const.tile([4, QT], F32, tag="rstd4")
            std4 = const.tile([4, QT], F32, tag="std4")
            std4b = const.tile([4, QT], BF16, tag="std4b")
            nmu4b = const.tile([4, QT], BF16, tag="nmu4b")
            mu4 = const.tile([4, QT], F32, tag="mu4")
            var4 = const.tile([4, QT], F32, tag="var4")
            musq4 = const.tile([4, QT], F32, tag="musq4")

            # ---------------- phase A: LN stats ----------------
            with (
                tc.tile_pool(name="ps_stats", bufs=1, space="PSUM") as pstat,
            ):
                # Two [4, 512] PSUM tiles: token-slice nt's sum / sum-of-sq
                # rows land on partition nt via the sel4 stationary (other
                # rows accumulate zero), one accumulation group per tile.
                sum_ps = pstat.tile([4, QT], F32, tag="sum")
                sq_ps = pstat.tile([4, QT], F32, tag="sq")
                for kc in range(NKD):
                    for nt in range(NQT):
                        sl = slice(nt * QT, (nt + 1) * QT)
                        first = kc == 0 and nt == 0
                        last = kc == NKD - 1 and nt == NQT - 1
                        sq_t = sqp.tile([128, QT], BF16, tag="sq_t")
                        nc.vector.tensor_mul(sq_t, xt[kc][:, sl], xt[kc][:, sl])
                        nc.tensor.matmul(
                            out=sum_ps, lhsT=sel4[:, 4 * nt:4 * nt + 4],
                            rhs=xt[kc][:, sl],
                            start=first, stop=last)
                        nc.tensor.matmul(
                            out=sq_ps, lhsT=sel4[:, 4 * nt:4 * nt + 4],
                            rhs=sq_t,
                            start=first, stop=last)
                # postproc on [4, 512] (4 lanes)
                nc.vector.tensor_scalar_mul(mu4, sum_ps, 1.0 / DIM)
                nc.vector.tensor_scalar_mul(var4, sq_ps, 1.0 / DIM)
                nc.vector.tensor_mul(musq4, mu4, mu4)
                nc.vector.tensor_sub(var4, var4, musq4)
                nc.scalar.activation(out=std4, in_=var4, func=ACT_SQRT,
                                     bias=eps4, scale=1.0)
                nc.vector.reciprocal(rstd4, std4)
                nc.vector.tensor_copy(std4b, std4)
                nc.vector.tensor_scalar_mul(nmu4b, mu4, -1.0)
                # gather LN rows into operand layouts (cross-shape DMAs:
                # only total element count must match)
                nc.sync.dma_start(out=onm[0:1, :], in_=std4b[:, :])
                nc.sync.dma_start(out=onm[1:2, :], in_=nmu4b[:, :])
                # rstd broadcast [128, tok]: bounce through DRAM, then a
                # stride-0 partition-broadcast read
                nc.sync.dma_start(out=rstd_d[0:1, :], in_=rstd4[:, :])
                nc.sync.dma_start(
                    out=bc_sb,
                    in_=bass.AP(tensor=rstd_d, offset=0,
                                ap=[[0, 128], [1, tok]]))
            # ------- phases B-D: projections + attention + out-proj -------
            # One shared [128, 512]-f32 PSUM pool ("proj") serves the QK
            # slices, the V groups, the rstd transposes and the out-proj
            # groups; they never overlap in time.  8 banks total:
            # proj 2 + scores 4 + o_ps 2.
            v_sb = [None] * NTT
            oTs = []
            with (
                tc.tile_pool(name="ps_proj", bufs=2, space="PSUM") as pproj,
                tc.tile_pool(name="ps_s", bufs=2, space="PSUM") as pss,
                tc.tile_pool(name="ps_o", bufs=2, space="PSUM") as pso,
                tc.tile_pool(name="out_sb", bufs=4) as outp,
            ):
                def emit_qk_slice(p, dst, d, nt):
                    """One [128, 512] token-slice of the Q or K projection of
                    pair p (d=0 -> Q, d=1 -> K).  Seed matmul last so the
                    group never waits on the LN postprocessing."""
                    sl = slice(nt * QT, (nt + 1) * QT)
                    cofs = d * INNER_PC + p * 128
                    ps = pproj.tile([128, QT], F32, tag="proj", name="qk_ps")
                    for kc in range(NKD):
                        nc.tensor.matmul(
                            out=ps, lhsT=wsb[kc][:, cofs:cofs + 128],
                            rhs=xt[kc][:, sl],
                            start=(kc == 0), stop=False)
                    nc.tensor.matmul(
                        out=ps, lhsT=seed_sb[:, cofs:cofs + 128],
                        rhs=onm[:, sl], start=False, stop=True)
                    nc.vector.tensor_mul(dst[:, sl], ps, bc_sb[:, sl])

                def emit_rstd_col():
                    # 4 PE transposes of [4, 128] blocks; block g lands at
                    # contiguous cols [4g, 4g+4) (permuted order, see above).
                    rc_ps = pproj.tile([128, QT], F32, tag="proj", name="rc")
                    for g in range(4):
                        nc.tensor.transpose(
                            out=rc_ps[:, 4 * g:4 * (g + 1)],
                            in_=rstd4[:, g * 128:(g + 1) * 128],
                            identity=eye4)
                    nc.vector.tensor_copy(rstd_col, rc_ps[:, 0:NTT])

                def emit_v_group(tt):
                    tsl = slice(tt * KC, (tt + 1) * KC)
                    v_ps = pproj.tile([128, INNER_PC], F32, tag="proj",
                                      name="v_ps")
                    for kc in range(NKD):
                        nc.tensor.matmul(
                            out=v_ps, lhsT=xt[kc][:, tsl],
                            rhs=wsb[kc][:, 2 * INNER_PC:3 * INNER_PC],
                            start=(kc == 0), stop=False)
                    nc.tensor.matmul(
                        out=v_ps, lhsT=onm[:, tsl],
                        rhs=seed_sb[:, 2 * INNER_PC:3 * INNER_PC],
                        start=False, stop=True)
                    vt = vpool.tile([128, HEADS_PC * (DH + 1)], BF16,
                                    tag="v_sb", name=f"v_sb{tt}")
                    v3 = vt.rearrange("p (h w) -> p h w", w=DH + 1)
                    pc = 4 * (tt % 4) + tt // 4  # permuted rstd_col index
                    nc.vector.tensor_scalar(
                        out=v3[:, :, 0:DH],
                        in0=v_ps.rearrange("p (h w) -> p h w", w=DH),
                        scalar1=rstd_col[:, pc:pc + 1], scalar2=None,
                        op0=ALU.mult)
                    nc.vector.memset(v3[:, :, DH:DH + 1], 1.0)
                    v_sb[tt] = vt

                def emit_outproj_tt(tt):
                    tsl = slice(tt * KC, (tt + 1) * KC)
                    for nb in range(DIM // QT):
                        nsl = slice(nb * QT, (nb + 1) * QT)
                        ps = pproj.tile([128, QT], F32, tag="proj",
                                        name="out_ps")
                        for p in range(NPAIRS):
                            nc.tensor.matmul(
                                out=ps, lhsT=oTs[p][:, tsl],
                                rhs=wos[p][:, nsl],
                                start=(p == 0), stop=(p == NPAIRS - 1))
                        ob = outp.tile([128, QT], F32, tag="out_sb")
                        nc.vector.tensor_copy(ob, ps)
                        nc.sync.dma_start(out=out[tsl, nsl], in_=ob)

                def emit_attn_qtile(p, t_i, qT, kT, oT):
                    """Scores/exp/mask/PV for one query tile, then the
                    per-qtile denominator chain and normalize into oT."""
                    qsl0 = t_i * QT
                    nch = (t_i + 1) * QT // KC
                    o_ps = [pso.tile([DH + 1, QT], F32, tag="o_ps",
                                     name=f"o_ps{p}_{t_i}_{h}")
                            for h in range(2)]
                    p_tiles = {}

                    def emit_scores(c):
                        m = c - (nch - 4)
                        lo = 128 * m if m > 0 else 0
                        csl = slice(c * KC, (c + 1) * KC)
                        s_ps = pss.tile([128, 2 * QT], F32, tag="s_ps")
                        p_sb = ppool.tile([128, 2 * QT], BF16, tag="p_sb")
                        for h in range(2):
                            nc.tensor.matmul(
                                out=s_ps[:, h * QT + lo:(h + 1) * QT],
                                lhsT=kT[h * DH:(h + 1) * DH, csl],
                                rhs=qT[h * DH:(h + 1) * DH,
                                       qsl0 + lo:qsl0 + QT],
                                start=True, stop=True)
                        s3 = s_ps.rearrange("p (h q) -> p h q", q=QT)
                        p3 = p_sb.rearrange("p (h q) -> p h q", q=QT)
                        nc.scalar.activation(
                            out=p3[:, :, lo:QT], in_=s3[:, :, lo:QT],
                            func=ACT_EXP, scale=SCALE)
                        if m >= 0:
                            for h in range(2):
                                nc.vector.tensor_mul(
                                    p_sb[:, h * QT + lo:h * QT + lo + KC],
                                    p_sb[:, h * QT + lo:h * QT + lo + KC],
                                    mask_sb)
                        p_tiles[c] = p_sb

                    def emit_pv(c):
                        m = c - (nch - 4)
                        lo = 128 * m if m > 0 else 0
                        p_sb = p_tiles.pop(c)
                        for h in range(2):
                            hc = (2 * p + h) * (DH + 1)
                            nc.tensor.matmul(
                                out=o_ps[h][:, lo:QT],
                                lhsT=v_sb[c][:, hc:hc + DH + 1],
                                rhs=p_sb[:, h * QT + lo:(h + 1) * QT],
                                start=(c == 0), stop=(c == nch - 1),
                                skip_group_check=True)

                    emit_scores(0)
                    for c in range(1, nch):
                        emit_scores(c)
                        emit_pv(c - 1)
                    emit_pv(nch - 1)
                    # free PSUM fast (O^T rows + denominator row 64)
                    o_sb = [osbp.tile([DH + 1, QT], BF16, tag=f"o_sb{h}",
                                      name=f"o_sb{p}_{t_i}_{h}")
                            for h in range(2)]
                    for h in range(2):
                        nc.vector.tensor_copy(o_sb[h], o_ps[h])
                    # per-qtile denominator chain: den16 partition 8h+j
                    # holds tokens [64j, 64j+64) of head h; flattened
                    # partition-major this gives rec_d offsets 512h+64j+e.
                    den16 = denp.tile([16, 64], BF16, tag="den16")
                    rec16 = denp.tile([16, 64], BF16, tag="rec16")
                    rb_q = denp.tile([64, 2 * QT], BF16, tag="rb_q")
                    for h in range(2):
                        nc.sync.dma_start(
                            out=den16[8 * h:8 * h + 8, :],
                            in_=o_sb[h][DH:DH + 1, :])
                    nc.vector.reciprocal(rec16, den16)
                    dofs = p * 8 * QT + t_i * 2 * QT
                    nc.sync.dma_start(
                        out=bass.AP(tensor=rec_d, offset=dofs,
                                    ap=[[2 * QT, 1], [1, 2 * QT]]),
                        in_=rec16[:, :])
                    nc.sync.dma_start(
                        out=rb_q,
                        in_=bass.AP(tensor=rec_d, offset=dofs,
                                    ap=[[0, 64], [1, 2 * QT]]))
                    qsl = slice(qsl0, qsl0 + QT)
                    for h in range(2):
                        nc.vector.tensor_mul(
                            oT[h * DH:(h + 1) * DH, qsl],
                            o_sb[h][0:DH, :],
                            rb_q[:, h * QT:(h + 1) * QT])

                # ---- schedule ----
                qT = qkp.tile([128, tok], BF16, tag="qT", name="qT0")
                kT = qkp.tile([128, tok], BF16, tag="kT", name="kT0")
                emit_qk_slice(0, qT, 0, 0)
                emit_qk_slice(0, kT, 1, 0)
                emit_rstd_col()
                for tt in range(4):
                    emit_v_group(tt)

                for p in range(NPAIRS):
                    oT = oTp.tile([128, tok], BF16, tag="oT", name=f"oT{p}")
                    oTs.append(oT)
                    nxt = []
                    if p + 1 < NPAIRS:
                        qT2 = qkp.tile([128, tok], BF16, tag="qT",
                                       name=f"qT{p + 1}")
                        kT2 = qkp.tile([128, tok], BF16, tag="kT",
                                       name=f"kT{p + 1}")
                        nxt = [(p + 1, dst, d, nt)
                               for d, dst in ((0, qT2), (1, kT2))
                               for nt in range(NQT)]
                    for t_i in range(NQT):
                        emit_attn_qtile(p, t_i, qT, kT, oT)
                        if p == 0 and t_i < 3:
                            # just-in-time rest of pair 0's QK and V
                            emit_qk_slice(0, qT, 0, t_i + 1)
                            emit_qk_slice(0, kT, 1, t_i + 1)
                            for tt in range(4 * (t_i + 1), 4 * (t_i + 2)):
                                emit_v_group(tt)
                        if p == NPAIRS - 1:
                            # out-proj for this qtile's tokens rides along
                            for tt in range(4 * t_i, 4 * (t_i + 1)):
                                emit_outproj_tt(tt)
                        for _ in range(2):
                            if nxt:
                                emit_qk_slice(*nxt.pop(0))
                    while nxt:
                        emit_qk_slice(*nxt.pop(0))
                    if p + 1 < NPAIRS:
                        qT, kT = qT2, kT2

    return nc


def make_masks():
    import ml_dtypes

    k = np.arange(KC)[:, None]
    q = np.arange(KC)[None, :]
    return (q >= k).astype(ml_dtypes.bfloat16)


def make_in_maps(x, ln_gamma, ln_beta, w_qkv, w_out):
    import ml_dtypes

    bf16 = ml_dtypes.bfloat16
    x = np.asarray(x, np.float32)
    g_ = np.asarray(ln_gamma, np.float32)
    b_ = np.asarray(ln_beta, np.float32)
    w_qkv = np.asarray(w_qkv, np.float32)
    w_out = np.asarray(w_out, np.float32)
    mask128 = make_masks()
    eye4 = np.eye(4, dtype=np.float32)
    in_maps = []
    for c in range(8):
        b = c // 2
        g = c % 2
        cs = slice(g * INNER_PC, (g + 1) * INNER_PC)
        Wraw = np.concatenate(
            [w_qkv[:, 0 * DIM:1 * DIM][:, cs],
             w_qkv[:, 1 * DIM:2 * DIM][:, cs],
             w_qkv[:, 2 * DIM:3 * DIM][:, cs]], axis=1)
        Wp = (Wraw * g_[:, None]).astype(bf16)
        seed = np.stack([b_ @ Wraw,
                         Wp.astype(np.float32).sum(axis=0)]).astype(bf16)
        in_maps.append({
            "xT": np.ascontiguousarray(x[b].T).astype(bf16),
            "w": np.ascontiguousarray(Wp),
            "seed": seed,
            "wo": np.ascontiguousarray(w_out[cs, :]).astype(bf16),
            "mask128": mask128,
            "eye4": eye4,
        })
    return in_maps


_PROG = None


def kernel(x, ln_gamma, ln_beta, w_qkv, w_out):
    global _PROG
    from concourse.bass_utils import run_bass_kernel_spmd

    if _PROG is None:
        _PROG = build_program(TOK)
    in_maps = make_in_maps(x, ln_gamma, ln_beta, w_qkv, w_out)
    res = run_bass_kernel_spmd(_PROG, in_maps, list(range(8)))
    parts = [res.results[c]["out"] for c in range(8)]
    out = np.empty((B, TOK, DIM), np.float32)
    for b in range(B):
        out[b] = parts[2 * b] + parts[2 * b + 1]
    return out


# revision 35
# speedup vs baseline: 1.8586x; 1.0202x over previous
"""Causal attention block (LN -> QKV -> causal MHA -> out-proj) on 8 trn2
NeuronCores via Bass/Tile.

Sharding: core c handles batch b=c//2 and head-group g=c%2 (8 of 16 heads).
Data parallel over batch, tensor parallel over heads; the out-proj partial
sums (2 per batch) are reduced on the host during the gather, so the device
program needs no collectives and is pure SPMD.

v2 design (bf16 everywhere, PE kept warm, no DRAM roundtrips):
  - all matmul operands bf16 (full-rate 1 cyc/row, FWL weight loads, half
    the DMA); PSUM accumulation stays fp32.
  - LN stats land in a [4, 512] PSUM layout so the postprocessing runs on
    4 partitions instead of 1; rstd is folded into the PSUM->SBUF copies
    (per-partition tensor_scalar for V, a broadcast-row multiply for Q/K),
    so x itself is never rewritten.  The mean/beta correction enters each
    projection as a K=2 seed matmul with rows [std, -mu].
  - scores are computed transposed per 128-key chunk with 2 heads packed;
    exp runs on ACT straight out of PSUM with the 1/sqrt(dh) scale folded
    in; diagonal chunks are narrowed to the valid q-range (saves PE + ACT)
    and only the 128x128 boundary block gets a 0/1 mask multiply.
  - softmax denominators ride as a 65th ones-column of V through the P@V
    matmul; per pair they are DMA-gathered into a [64, 64] tile for one
    batched reciprocal, spread back with two small DMAs, and applied in the
    normalize copy into the SBUF-resident O^T (no DRAM scratch).
  - the QK projection of pair p+1 is interleaved into the (ACT-bound)
    attention stream of pair p so the PE never idles long enough to be
    clock-throttled.
"""

import numpy as np

import concourse.bass as bass
import concourse.mybir as mybir
import concourse.tile as tile_mod

# ----------------------------------------------------------------------------
# Workaround for this walrus build rejecting instructions that carry more than
# MAX_WAITS semaphore waits ("Too many sync wait commands" in CoreV3GenImpl
# setupSyncWait — hit on Drain and Matmult/S3_LW encodings). Split excess
# waits onto single-wait NOP carrier instructions emitted just before the
# original instruction on the same engine: program order on the sequencer
# makes this semantically identical (waits are AND conditions).
# ----------------------------------------------------------------------------
_MAX_WAITS = 1
_orig_add_instruction = tile_mod.TileContext._add_instruction
_carrier_id = [0]


def _split_waits_add_instruction(self, inst):
    si = inst.sync_info
    if (
        si is not None
        and si.on_wait
        and len(si.on_wait) > _MAX_WAITS
        and inst.engine != mybir.EngineType.Unassigned
    ):
        waits = list(si.on_wait)
        keep = waits[-_MAX_WAITS:]
        for w in waits[:-_MAX_WAITS]:
            _carrier_id[0] += 1
            nop = mybir.InstNoOp(name=f"I-waitc-{_carrier_id[0]}")
            nop.engine = inst.engine
            nop.sync_info = mybir.SyncInfo(on_wait=[w], on_update=[])
            _orig_add_instruction(self, nop)
        inst.sync_info = mybir.SyncInfo(
            on_wait=keep,
            on_update=list(si.on_update) if si.on_update else [],
        )
    _orig_add_instruction(self, inst)


tile_mod.TileContext._add_instruction = _split_waits_add_instruction

from concourse.vector_clock import ScopedClock


def _patched_drain_and_barrier(self, tick_clock, wait_clock):
    # Same wait-splitting for the TileContext exit drain, which is emitted
    # after lowering (outside _add_instruction).
    nc = self.nc
    carrier = nc.sync.nop(nofuse=True)
    wait_clock.add_sem_waits(carrier.ins, ScopedClock({None: tick_clock.global_clock}))
    si = carrier.ins.sync_info
    waits = list(si.on_wait) if si is not None and si.on_wait else []
    if len(waits) > _MAX_WAITS:
        carrier.ins.sync_info = mybir.SyncInfo(
            on_wait=waits[:_MAX_WAITS],
            on_update=list(si.on_update) if si.on_update else [],
        )
        rest = waits[_MAX_WAITS:]
        while rest:
            extra = nc.sync.nop(nofuse=True)
            extra.ins.sync_info = mybir.SyncInfo(
                on_wait=rest[:_MAX_WAITS], on_update=[])
            rest = rest[_MAX_WAITS:]

    nc.sync.drain()
    nc.all_engine_barrier()
    assert self.sems is not None
    popped = nc._tile_sem_poison_stack.pop()
    assert popped is self._sem_poison
    nc.clear_and_free_semaphores(list(self.sems.allocated().values()))
    nc.all_engine_barrier()


tile_mod.TileContext._drain_and_barrier = _patched_drain_and_barrier

# ----------------------------------------------------------------------------

F32 = mybir.dt.float32
BF16 = mybir.dt.bfloat16
ALU = mybir.AluOpType
ACT_EXP = mybir.ActivationFunctionType.Exp
ACT_SQRT = mybir.ActivationFunctionType.Sqrt

B = 4
TOK = 2048
DIM = 1024
HEADS = 16
DH = 64
HEADS_PC = 8          # heads per core
INNER_PC = HEADS_PC * DH  # 512
NPAIRS = HEADS_PC // 2
QT = 512              # query tile
KC = 128              # key-token chunk
NQT = TOK // QT       # 4
NTT = TOK // KC       # 16
NKD = DIM // 128      # 8
EPS = 1e-5
SCALE = DH ** -0.5


def _ap0(ap, parts):
    """Partition-broadcast AP (stride-0 leading dim) for DMA reads."""
    return bass.AP(tensor=ap.tensor, offset=ap.offset, ap=[[0, parts]] + list(ap.ap))


def _pstride(t, step, n):
    """View tile t's partitions with a stride (partition p -> p*step)."""
    return bass.AP(tensor=t.tensor, offset=t.offset,
                   ap=[[t.ap[0][0] * step, n]] + list(t.ap[1:]))


def build_program(tok=TOK):
    nc = bass.Bass()
    xT = nc.declare_dram_parameter("xT", [DIM, tok], BF16, isOutput=False)
    w = nc.declare_dram_parameter("w", [DIM, 3 * INNER_PC], BF16, isOutput=False)
    seed = nc.declare_dram_parameter("seed", [2, 3 * INNER_PC], BF16, isOutput=False)
    wo = nc.declare_dram_parameter("wo", [INNER_PC, DIM], BF16, isOutput=False)
    mask_d = nc.declare_dram_parameter("mask128", [KC, KC], BF16, isOutput=False)
    eye_d = nc.declare_dram_parameter("eye4", [4, 4], F32, isOutput=False)
    out = nc.declare_dram_parameter("out", [tok, DIM], F32, isOutput=True)
    # DRAM bounce buffers for partition-broadcasts (stride-0 reads are only
    # legal on the DRAM side of a DMA)
    rstd_d = nc.dram_tensor("rstd_row", [1, tok], F32)
    rec_d = nc.dram_tensor("rec_row", [NPAIRS, 8 * QT], BF16)

    with tile_mod.TileContext(nc) as tc, nc.allow_low_precision(
            "bf16 operand tiles; all matmul accumulation stays fp32 PSUM"):
        with (
            tc.tile_pool(name="const", bufs=1) as const,
            tc.tile_pool(name="xt", bufs=NKD) as xtp,
            tc.tile_pool(name="wsb", bufs=NKD) as wp,
            tc.tile_pool(name="wosb", bufs=NPAIRS) as wop,
            tc.tile_pool(name="vpool", bufs=NTT) as vpool,
            tc.tile_pool(name="qkT", bufs=2) as qkp,
            tc.tile_pool(name="osb", bufs=3) as osbp,
            tc.tile_pool(name="oT", bufs=NPAIRS) as oTp,
            tc.tile_pool(name="psb", bufs=3) as ppool,
            tc.tile_pool(name="den", bufs=2) as denp,
            tc.tile_pool(name="sq", bufs=2) as sqp,
            tc.tile_pool(name="out_sb", bufs=4) as outp,
        ):
            # ---------------- constants / big loads ----------------
            ones_col = const.tile([128, 1], BF16, tag="ones_col")
            nc.vector.memset(ones_col, 1.0)
            # sel4: 4 stationary variants [128, 4]; variant v is all-ones in
            # column v, zero elsewhere.  Routes LN stat rows to consecutive
            # PSUM partitions (matmul/DVE PSUM bases must be 32-aligned).
            sel4 = const.tile([128, 16], BF16, tag="sel4")
            nc.vector.memset(sel4, 0.0)
            for v in range(4):
                nc.vector.memset(sel4[:, 5 * v:5 * v + 1], 1.0)
            eps4 = const.tile([4, 1], F32, tag="eps")
            nc.vector.memset(eps4, EPS)
            mask_sb = const.tile([KC, KC], BF16, tag="mask")
            nc.sync.dma_start(out=mask_sb, in_=mask_d[:, :])
            eye4 = const.tile([4, 4], F32, tag="eye4")
            nc.sync.dma_start(out=eye4, in_=eye_d[:, :])
            seed_sb = const.tile([2, 3 * INNER_PC], BF16, tag="seed")
            nc.sync.dma_start(out=seed_sb, in_=seed[:, :])

            xt = []
            for kc in range(NKD):
                t = xtp.tile([128, tok], BF16, tag="xt")
                nc.sync.dma_start(out=t, in_=xT[kc * 128:(kc + 1) * 128, :])
                xt.append(t)
            wsb = []
            for kc in range(NKD):
                t = wp.tile([128, 3 * INNER_PC], BF16, tag="wsb")
                nc.sync.dma_start(out=t, in_=w[kc * 128:(kc + 1) * 128, :])
                wsb.append(t)
            wos = []
            for p in range(NPAIRS):
                t = wop.tile([128, DIM], BF16, tag="wosb")
                nc.sync.dma_start(out=t, in_=wo[p * 128:(p + 1) * 128, :])
                wos.append(t)

            # LN-derived rows (device computed, long-lived)
            onm = const.tile([2, tok], BF16, tag="onm")        # r0=std r1=-mu
            bc_sb = const.tile([128, tok], F32, tag="bc")      # rstd bcast
            # NOTE: rstd_col columns are block-permuted: chunk tt lives at
            # column 4*(tt%4) + tt//4 (transpose blocks land contiguously).
            rstd_col = const.tile([128, NTT], F32, tag="rstdc")
            rstd4 = const.tile([4, QT], F32, tag="rstd4")
            std4 = const.tile([4, QT], F32, tag="std4")
            std4b = const.tile([4, QT], BF16, tag="std4b")
            nmu4b = const.tile([4, QT], BF16, tag="nmu4b")
            mu4 = const.tile([4, QT], F32, tag="mu4")
            var4 = const.tile([4, QT], F32, tag="var4")
            musq4 = const.tile([4, QT], F32, tag="musq4")

            # ---------------- phase A: LN stats ----------------
            with (
                tc.tile_pool(name="ps_stats", bufs=1, space="PSUM") as pstat,
            ):
                # Warm-up: ~4us of dummy matmuls on constants while the x
                # DMAs land, so the HAM clock gate releases (1.2 -> 2.4 GHz)
                # before the real work starts.
                warm_ps = pstat.tile([4, KC], F32, tag="warm")
                for _ in range(40):
                    nc.tensor.matmul(out=warm_ps, lhsT=sel4[:, 0:4],
                                     rhs=mask_sb, start=True, stop=True)
                # Two [4, 512] PSUM tiles: token-slice nt's sum / sum-of-sq
                # rows land on partition nt via the sel4 stationary (other
                # rows accumulate zero), one accumulation group per tile.
                sum_ps = pstat.tile([4, QT], F32, tag="sum")
                sq_ps = pstat.tile([4, QT], F32, tag="sq")
                for kc in range(NKD):
                    for nt in range(NQT):
                        sl = slice(nt * QT, (nt + 1) * QT)
                        first = kc == 0 and nt == 0
                        last = kc == NKD - 1 and nt == NQT - 1
                        sq_t = sqp.tile([128, QT], BF16, tag="sq_t")
                        nc.vector.tensor_mul(sq_t, xt[kc][:, sl], xt[kc][:, sl])
                        nc.tensor.matmul(
                            out=sum_ps, lhsT=sel4[:, 4 * nt:4 * nt + 4],
                            rhs=xt[kc][:, sl],
                            start=first, stop=last)
                        nc.tensor.matmul(
                            out=sq_ps, lhsT=sel4[:, 4 * nt:4 * nt + 4],
                            rhs=sq_t,
                            start=first, stop=last)
                # postproc on [4, 512] (4 lanes)
                nc.vector.tensor_scalar_mul(mu4, sum_ps, 1.0 / DIM)
                nc.vector.tensor_scalar_mul(var4, sq_ps, 1.0 / DIM)
                nc.vector.tensor_mul(musq4, mu4, mu4)
                nc.vector.tensor_sub(var4, var4, musq4)
                nc.scalar.activation(out=std4, in_=var4, func=ACT_SQRT,
                                     bias=eps4, scale=1.0)
                nc.vector.reciprocal(rstd4, std4)
                nc.vector.tensor_copy(std4b, std4)
                nc.vector.tensor_scalar_mul(nmu4b, mu4, -1.0)
                # gather LN rows into operand layouts (cross-shape DMAs:
                # only total element count must match)
                # rstd broadcast [128, tok]: bounce through DRAM, then
                # stride-0 partition-broadcast reads (split per 512-slice so
                # the first Q/K copies unblock as early as possible)
                nc.sync.dma_start(out=rstd_d[0:1, :], in_=rstd4[:, :])
                nc.sync.dma_start(out=onm[0:1, :], in_=std4b[:, :])
                nc.sync.dma_start(out=onm[1:2, :], in_=nmu4b[:, :])
                for j in range(NQT):
                    nc.sync.dma_start(
                        out=bc_sb[:, j * QT:(j + 1) * QT],
                        in_=bass.AP(tensor=rstd_d, offset=j * QT,
                                    ap=[[0, 128], [1, QT]]))
            # ------- phases B-D: projections + attention + out-proj -------
            # One shared [128, 512]-f32 PSUM pool ("proj") serves the QK
            # slices, the V groups, the rstd transposes and the out-proj
            # groups; they never overlap in time.  8 banks total:
            # proj 2 + scores 4 + o_ps 2.
            v_sb = [None] * NTT
            oTs = []
            with (
                tc.tile_pool(name="ps_proj", bufs=2, space="PSUM") as pproj,
                tc.tile_pool(name="ps_s", bufs=2, space="PSUM") as pss,
                tc.tile_pool(name="ps_o", bufs=2, space="PSUM") as pso,
            ):
                def emit_qk_slice(p, dst, d, nt):
                    """One [128, 512] token-slice of the Q or K projection of
                    pair p (d=0 -> Q, d=1 -> K).  Seed matmul last so the
                    group never waits on the LN postprocessing."""
                    sl = slice(nt * QT, (nt + 1) * QT)
                    cofs = d * INNER_PC + p * 128
                    ps = pproj.tile([128, QT], F32, tag="proj", name="qk_ps")
                    for kc in range(NKD):
                        nc.tensor.matmul(
                            out=ps, lhsT=wsb[kc][:, cofs:cofs + 128],
                            rhs=xt[kc][:, sl],
                            start=(kc == 0), stop=False)
                    nc.tensor.matmul(
                        out=ps, lhsT=seed_sb[:, cofs:cofs + 128],
                        rhs=onm[:, sl], start=False, stop=True)
                    nc.vector.tensor_mul(dst[:, sl], ps, bc_sb[:, sl])

                def emit_rstd_col():
                    # 4 PE transposes of [4, 128] blocks; block g lands at
                    # contiguous cols [4g, 4g+4) (permuted order, see above).
                    rc_ps = pproj.tile([128, QT], F32, tag="proj", name="rc")
                    for g in range(4):
                        nc.tensor.transpose(
                            out=rc_ps[:, 4 * g:4 * (g + 1)],
                            in_=rstd4[:, g * 128:(g + 1) * 128],
                            identity=eye4)
                    nc.vector.tensor_copy(rstd_col, rc_ps[:, 0:NTT])

                def emit_v_group(tt):
                    tsl = slice(tt * KC, (tt + 1) * KC)
                    v_ps = pproj.tile([128, INNER_PC], F32, tag="proj",
                                      name="v_ps")
                    for kc in range(NKD):
                        nc.tensor.matmul(
                            out=v_ps, lhsT=xt[kc][:, tsl],
                            rhs=wsb[kc][:, 2 * INNER_PC:3 * INNER_PC],
                            start=(kc == 0), stop=False)
                    nc.tensor.matmul(
                        out=v_ps, lhsT=onm[:, tsl],
                        rhs=seed_sb[:, 2 * INNER_PC:3 * INNER_PC],
                        start=False, stop=True)
                    vt = vpool.tile([128, HEADS_PC * (DH + 1)], BF16,
                                    tag="v_sb", name=f"v_sb{tt}")
                    v3 = vt.rearrange("p (h w) -> p h w", w=DH + 1)
                    pc = 4 * (tt % 4) + tt // 4  # permuted rstd_col index
                    nc.vector.tensor_scalar(
                        out=v3[:, :, 0:DH],
                        in0=v_ps.rearrange("p (h w) -> p h w", w=DH),
                        scalar1=rstd_col[:, pc:pc + 1], scalar2=None,
                        op0=ALU.mult)
                    nc.vector.memset(v3[:, :, DH:DH + 1], 1.0)
                    v_sb[tt] = vt

                def emit_outproj_tt(tt):
                    tsl = slice(tt * KC, (tt + 1) * KC)
                    for nb in range(DIM // QT):
                        nsl = slice(nb * QT, (nb + 1) * QT)
                        ps = pproj.tile([128, QT], F32, tag="proj",
                                        name="out_ps")
                        for p in range(NPAIRS):
                            nc.tensor.matmul(
                                out=ps, lhsT=oTs[p][:, tsl],
                                rhs=wos[p][:, nsl],
                                start=(p == 0), stop=(p == NPAIRS - 1))
                        ob = outp.tile([128, QT], F32, tag="out_sb")
                        nc.vector.tensor_copy(ob, ps)
                        nc.sync.dma_start(out=out[tsl, nsl], in_=ob)

                def emit_attn_qtile(p, t_i, qT, kT, oT):
                    """Scores/exp/mask/PV for one query tile, then the
                    per-qtile denominator chain and normalize into oT."""
                    qsl0 = t_i * QT
                    nch = (t_i + 1) * QT // KC
                    o_ps = [pso.tile([DH + 1, QT], F32, tag="o_ps",
                                     name=f"o_ps{p}_{t_i}_{h}")
                            for h in range(2)]
                    p_tiles = {}

                    def emit_scores(c):
                        m = c - (nch - 4)
                        lo = 128 * m if m > 0 else 0
                        csl = slice(c * KC, (c + 1) * KC)
                        s_ps = pss.tile([128, 2 * QT], F32, tag="s_ps")
                        p_sb = ppool.tile([128, 2 * QT], BF16, tag="p_sb")
                        for h in range(2):
                            nc.tensor.matmul(
                                out=s_ps[:, h * QT + lo:(h + 1) * QT],
                                lhsT=kT[h * DH:(h + 1) * DH, csl],
                                rhs=qT[h * DH:(h + 1) * DH,
                                       qsl0 + lo:qsl0 + QT],
                                start=True, stop=True)
                        s3 = s_ps.rearrange("p (h q) -> p h q", q=QT)
                        p3 = p_sb.rearrange("p (h q) -> p h q", q=QT)
                        nc.scalar.activation(
                            out=p3[:, :, lo:QT], in_=s3[:, :, lo:QT],
                            func=ACT_EXP, scale=SCALE)
                        if m >= 0:
                            for h in range(2):
                                nc.vector.tensor_mul(
                                    p_sb[:, h * QT + lo:h * QT + lo + KC],
                                    p_sb[:, h * QT + lo:h * QT + lo + KC],
                                    mask_sb)
                        p_tiles[c] = p_sb

                    def emit_pv(c):
                        m = c - (nch - 4)
                        lo = 128 * m if m > 0 else 0
                        p_sb = p_tiles.pop(c)
                        for h in range(2):
                            hc = (2 * p + h) * (DH + 1)
                            nc.tensor.matmul(
                                out=o_ps[h][:, lo:QT],
                                lhsT=v_sb[c][:, hc:hc + DH + 1],
                                rhs=p_sb[:, h * QT + lo:(h + 1) * QT],
                                start=(c == 0), stop=(c == nch - 1),
                                skip_group_check=True)

                    emit_scores(0)
                    for c in range(1, nch):
                        emit_scores(c)
                        emit_pv(c - 1)
                    emit_pv(nch - 1)
                    # free PSUM fast (O^T rows + denominator row 64)
                    o_sb = [osbp.tile([DH + 1, QT], BF16, tag=f"o_sb{h}",
                                      name=f"o_sb{p}_{t_i}_{h}")
                            for h in range(2)]
                    for h in range(2):
                        nc.vector.tensor_copy(o_sb[h], o_ps[h])
                    # per-qtile denominator chain: den16 partition 8h+j
                    # holds tokens [64j, 64j+64) of head h; flattened
                    # partition-major this gives rec_d offsets 512h+64j+e.
                    den16 = denp.tile([16, 64], BF16, tag="den16")
                    rec16 = denp.tile([16, 64], BF16, tag="rec16")
                    rb_q = denp.tile([64, 2 * QT], BF16, tag="rb_q")
                    for h in range(2):
                        nc.sync.dma_start(
                            out=den16[8 * h:8 * h + 8, :],
                            in_=o_sb[h][DH:DH + 1, :])
                    nc.vector.reciprocal(rec16, den16)
                    dofs = p * 8 * QT + t_i * 2 * QT
                    nc.sync.dma_start(
                        out=bass.AP(tensor=rec_d, offset=dofs,
                                    ap=[[2 * QT, 1], [1, 2 * QT]]),
                        in_=rec16[:, :])
                    nc.sync.dma_start(
                        out=rb_q,
                        in_=bass.AP(tensor=rec_d, offset=dofs,
                                    ap=[[0, 64], [1, 2 * QT]]))
                    qsl = slice(qsl0, qsl0 + QT)
                    for h in range(2):
                        nc.vector.tensor_mul(
                            oT[h * DH:(h + 1) * DH, qsl],
                            o_sb[h][0:DH, :],
                            rb_q[:, h * QT:(h + 1) * QT])

                # ---- schedule ----
                qT = qkp.tile([128, tok], BF16, tag="qT", name="qT0")
                kT = qkp.tile([128, tok], BF16, tag="kT", name="kT0")
                emit_qk_slice(0, qT, 0, 0)
                emit_qk_slice(0, kT, 1, 0)
                emit_rstd_col()
                for tt in range(4):
                    emit_v_group(tt)

                for p in range(NPAIRS):
                    oT = oTp.tile([128, tok], BF16, tag="oT", name=f"oT{p}")
                    oTs.append(oT)
                    nxt = []
                    if p + 1 < NPAIRS:
                        qT2 = qkp.tile([128, tok], BF16, tag="qT",
                                       name=f"qT{p + 1}")
                        kT2 = qkp.tile([128, tok], BF16, tag="kT",
                                       name=f"kT{p + 1}")
                        nxt = [(p + 1, dst, d, nt)
                               for d, dst in ((0, qT2), (1, kT2))
                               for nt in range(NQT)]
                    for t_i in range(NQT):
                        emit_attn_qtile(p, t_i, qT, kT, oT)
                        if p == 0 and t_i < 3:
                            # just-in-time rest of pair 0's QK and V
                            emit_qk_slice(0, qT, 0, t_i + 1)
                            emit_qk_slice(0, kT, 1, t_i + 1)
                            for tt in range(4 * (t_i + 1), 4 * (t_i + 2)):
                                emit_v_group(tt)
                        if p == NPAIRS - 1 and t_i < NQT - 1:
                            # out-proj for this qtile's tokens rides along
                            # (the last qtile's blocks run after the pools
                            # close, pairs 0-2 first, so they overlap the
                            # final denominator chain)
                            for tt in range(4 * t_i, 4 * (t_i + 1)):
                                emit_outproj_tt(tt)
                        for _ in range(2):
                            if nxt:
                                emit_qk_slice(*nxt.pop(0))
                    while nxt:
                        emit_qk_slice(*nxt.pop(0))
                    if p + 1 < NPAIRS:
                        qT, kT = qT2, kT2

            # Final out-proj blocks (tokens of the last qtile): pairs 0-2
            # accumulate while pair 3's last denominator chain completes,
            # the pair-3 matmul joins last.
            with tc.tile_pool(name="ps_fin", bufs=6, space="PSUM") as pfin:
                fin = [(tt, nb) for tt in range(4 * (NQT - 1), NTT)
                       for nb in range(DIM // QT)]
                tiles = {}

                def fin_p012(i):
                    tt, nb = fin[i]
                    ps = pfin.tile([128, QT], F32, tag="fin",
                                   name=f"fin{tt}_{nb}")
                    for p in range(NPAIRS - 1):
                        nc.tensor.matmul(
                            out=ps, lhsT=oTs[p][:, tt * KC:(tt + 1) * KC],
                            rhs=wos[p][:, nb * QT:(nb + 1) * QT],
                            start=(p == 0), stop=False)
                    tiles[i] = ps

                for i in range(6):
                    fin_p012(i)
                for i in range(len(fin)):
                    if i >= 6:
                        fin_p012(i)
                    tt, nb = fin[i]
                    nc.tensor.matmul(
                        out=tiles[i],
                        lhsT=oTs[NPAIRS - 1][:, tt * KC:(tt + 1) * KC],
                        rhs=wos[NPAIRS - 1][:, nb * QT:(nb + 1) * QT],
                        start=False, stop=True)
                    ob = outp.tile([128, QT], F32, tag="out_sb")
                    nc.vector.tensor_copy(ob, tiles.pop(i))
                    nc.sync.dma_start(
                        out=out[tt * KC:(tt + 1) * KC,
                                nb * QT:(nb + 1) * QT], in_=ob)

    return nc


def make_masks():
    import ml_dtypes

    k = np.arange(KC)[:, None]
    q = np.arange(KC)[None, :]
    return (q >= k).astype(ml_dtypes.bfloat16)


def make_in_maps(x, ln_gamma, ln_beta, w_qkv, w_out):
    import ml_dtypes

    bf16 = ml_dtypes.bfloat16
    x = np.asarray(x, np.float32)
    g_ = np.asarray(ln_gamma, np.float32)
    b_ = np.asarray(ln_beta, np.float32)
    w_qkv = np.asarray(w_qkv, np.float32)
    w_out = np.asarray(w_out, np.float32)
    mask128 = make_masks()
    eye4 = np.eye(4, dtype=np.float32)
    in_maps = []
    for c in range(8):
        b = c // 2
        g = c % 2
        cs = slice(g * INNER_PC, (g + 1) * INNER_PC)
        Wraw = np.concatenate(
            [w_qkv[:, 0 * DIM:1 * DIM][:, cs],
             w_qkv[:, 1 * DIM:2 * DIM][:, cs],
             w_qkv[:, 2 * DIM:3 * DIM][:, cs]], axis=1)
        Wp = (Wraw * g_[:, None]).astype(bf16)
        seed = np.stack([b_ @ Wraw,
                         Wp.astype(np.float32).sum(axis=0)]).astype(bf16)
        in_maps.append({
            "xT": np.ascontiguousarray(x[b].T).astype(bf16),
            "w": np.ascontiguousarray(Wp),
            "seed": seed,
            "wo": np.ascontiguousarray(w_out[cs, :]).astype(bf16),
            "mask128": mask128,
            "eye4": eye4,
        })
    return in_maps


_PROG = None


def kernel(x, ln_gamma, ln_beta, w_qkv, w_out):
    global _PROG
    from concourse.bass_utils import run_bass_kernel_spmd

    if _PROG is None:
        _PROG = build_program(TOK)
    in_maps = make_in_maps(x, ln_gamma, ln_beta, w_qkv, w_out)
    res = run_bass_kernel_spmd(_PROG, in_maps, list(range(8)))
    parts = [res.results[c]["out"] for c in range(8)]
    out = np.empty((B, TOK, DIM), np.float32)
    for b in range(B):
        out[b] = parts[2 * b] + parts[2 * b + 1]
    return out
